# revision 1
# baseline (speedup 1.0000x reference)
"""Tensor-parallel GQA attention kernel for 8 Trainium2 NeuronCores.

Problem: x[2,2048,2048] -> Attention(16 q heads, 4 kv heads, rotary,
causal) -> out[2,2048,2048].

Sharding: core c handles batch b=c//4 and kv-group g=c%4 (4 q-heads +
1 kv-head). Each core computes its heads' attention output and a
partial O-projection [DIM, S] (output-dim major); the host sums the 4
partials per batch and transposes.

On-core dataflow (everything feature/dim-major so matmul contractions
land on the partition axis). All matmul operands are fp16 (fp32 PSUM
accumulation): fp16 gets fast-weight-load so LDWEIGHTS hides under the
previous matmul's stream, and runs 1 cycle/row at any free size.
  xT = transpose(x) via PE-transpose (fp16)
  QT/KT/VT = W.T @ xT
  RoPE applied per 512-chunk right after projection (overlaps PE work).
  Weight columns are pair-permuted on the host so partitions 0..63
  hold "real" dims, 64..127 "imag".
  scoresT[k,q] = KT_tile.T @ QT (pairs of k-tiles into one 2-bank
  PSUM tile) -> one exp per pair (ACT, ->fp16) -> mask (diag chunks)
  outT[dv,q] += V_tile.T @ attnT, sums[1,q] += ones.T @ attnT
  normalize via batched reciprocal_approx_fast + K=1 broadcast-matmul
  OT[o,q] += wo_tile.T @ outT
"""
import numpy as np

import concourse.bass as bass
import concourse.tile as tile
import concourse.mybir as mybir
from concourse import bacc
from concourse import bass_utils

F32 = mybir.dt.float32
F32R = mybir.dt.float32r
F16 = mybir.dt.float16

DIM = 2048
S = 2048
B = 2
HL = 4           # q heads per core
FT = DIM // 128  # feature tiles
TT = S // 128    # token tiles
CH = 2           # token chunks (1024 tokens each) for projections
QC = 4           # q chunks (512) for attention
SCALE = 1.0 / np.sqrt(128.0)

_CACHE = {}


def _build():
    nc = bacc.Bacc("TRN2", target_bir_lowering=False, debug=False,
                   enable_asserts=True, num_devices=8)

    d_x = nc.dram_tensor("x_c", (S, DIM), F16, kind="ExternalInput").ap()
    d_wq = nc.dram_tensor("wq_c", (DIM, HL * 128), F16, kind="ExternalInput").ap()
    d_wk = nc.dram_tensor("wk_c", (DIM, 128), F16, kind="ExternalInput").ap()
    d_wv = nc.dram_tensor("wv_c", (DIM, 128), F16, kind="ExternalInput").ap()
    d_wo = nc.dram_tensor("wo_c", (HL * 128, DIM), F16, kind="ExternalInput").ap()
    d_cj = nc.dram_tensor("cjoin", (128, S), F16, kind="ExternalInput").ap()
    d_sj = nc.dram_tensor("sjoin", (128, S), F16, kind="ExternalInput").ap()
    d_mk = nc.dram_tensor("masks", (4, 128, 512), F16, kind="ExternalInput").ap()
    d_id = nc.dram_tensor("ident", (128, 128), F16, kind="ExternalInput").ap()
    d_ot = nc.dram_tensor("ot", (DIM, S), F32, kind="ExternalOutput").ap()

    Exp = mybir.ActivationFunctionType.Exp

    with tile.TileContext(nc) as tc:
        with tc.tile_pool(name="wts", bufs=1) as wp, \
             tc.tile_pool(name="acts", bufs=1) as ap:
            sb_id = wp.tile([128, 128], F16)
            nc.sync.dma_start(sb_id[:], d_id)
            sb_wq = wp.tile([128, FT, HL * 128], F16)
            nc.sync.dma_start(sb_wq[:], d_wq.rearrange("(ft p) m -> p ft m", p=128))
            sb_wk = wp.tile([128, FT, 128], F16)
            nc.sync.dma_start(sb_wk[:], d_wk.rearrange("(ft p) m -> p ft m", p=128))
            sb_wv = wp.tile([128, FT, 128], F16)
            nc.sync.dma_start(sb_wv[:], d_wv.rearrange("(ft p) m -> p ft m", p=128))
            sb_cj = wp.tile([128, S], F16)
            sb_sj = wp.tile([128, S], F16)
            sb_mk = wp.tile([128, 4, 512], F16)
            sb_wo = wp.tile([128, HL, DIM], F16)
            ones16 = wp.tile([128, 1], F16)
            nc.vector.memset(ones16[:], 1.0)
            ones32 = wp.tile([1, 128], F32)
            nc.vector.memset(ones32[:], 1.0)

            sb_QT = ap.tile([128, HL, S], F16)
            sb_KT = ap.tile([128, S], F16)
            sb_V = ap.tile([128, TT, 128], F16)
            sb_oT = ap.tile([128, HL, S], F16)

            # ---- Phase A: DMA-transpose x + Q/K/V projections + RoPE per chunk
            with tc.tile_pool(name="xT", bufs=2) as xT_p, \
                 tc.tile_pool(name="vt", bufs=2) as vt_p, \
                 tc.tile_pool(name="rope", bufs=2) as rp, \
                 tc.tile_pool(name="ps_tr", bufs=2, space="PSUM") as ps_tr, \
                 tc.tile_pool(name="ps_pj", bufs=3, space="PSUM") as ps_pj:

                def rope(T, c0):
                    # T: [128, 512] fp16 chunk at token offset c0
                    mc = rp.tile([128, 512], F16, tag="mc")
                    ms = rp.tile([128, 512], F16, tag="ms")
                    cjs = sb_cj[:, c0:c0 + 512]
                    sjs = sb_sj[:, c0:c0 + 512]
                    nc.gpsimd.tensor_mul(mc[:], T, cjs)
                    nc.vector.tensor_mul(ms[0:64, :], T[64:128, :], sjs[64:128, :])
                    nc.vector.tensor_mul(ms[64:128, :], T[0:64, :], sjs[0:64, :])
                    nc.vector.tensor_add(T, mc[:], ms[:])

                for ch in range(CH):
                    c0 = ch * 1024
                    xt = xT_p.tile([128, FT, 1024], F16)
                    for fi in range(FT):
                        nc.sync.dma_start(
                            xt[:, fi, :],
                            d_x[c0:c0 + 1024, fi * 128:(fi + 1) * 128],
                            transpose=True)
                    if ch == 0:
                        nc.scalar.dma_start(sb_cj[:], d_cj)
                        nc.scalar.dma_start(sb_sj[:], d_sj)
                        nc.scalar.dma_start(sb_mk[:], d_mk.rearrange("m p n -> p m n"))
                        nc.scalar.dma_start(sb_wo[:], d_wo.rearrange("(dv p) m -> p dv m", p=128))
                    for sc_ in range(2):
                        s0 = c0 + sc_ * 512
                        xts = xt[:, :, sc_ * 512:(sc_ + 1) * 512]
                        for h in range(HL):
                            pq = ps_pj.tile([128, 512], F32, tag="pj")
                            for fi in range(FT):
                                nc.tensor.matmul(
                                    pq[:], sb_wq[:, fi, h * 128:(h + 1) * 128],
                                    xts[:, fi, :], start=(fi == 0),
                                    stop=(fi == FT - 1))
                            nc.vector.tensor_copy(sb_QT[:, h, s0:s0 + 512], pq[:])
                            rope(sb_QT[:, h, s0:s0 + 512], s0)
                        pk = ps_pj.tile([128, 512], F32, tag="pj")
                        for fi in range(FT):
                            nc.tensor.matmul(pk[:], sb_wk[:, fi, :], xts[:, fi, :],
                                             start=(fi == 0), stop=(fi == FT - 1))
                        nc.vector.tensor_copy(sb_KT[:, s0:s0 + 512], pk[:])
                        rope(sb_KT[:, s0:s0 + 512], s0)
                        pv = ps_pj.tile([128, 512], F32, tag="pj")
                        for fi in range(FT):
                            nc.tensor.matmul(pv[:], sb_wv[:, fi, :], xts[:, fi, :],
                                             start=(fi == 0), stop=(fi == FT - 1))
                        vt = vt_p.tile([128, 512], F16)
                        nc.vector.tensor_copy(vt[:], pv[:])
                        for tl in range(4):
                            ti = (ch * 2 + sc_) * 4 + tl
                            ptv = ps_tr.tile([128, 128], F16, tag="tr")
                            nc.tensor.transpose(
                                ptv[:], vt[:, tl * 128:(tl + 1) * 128], sb_id[:])
                            nc.vector.tensor_copy(sb_V[:, ti, :], ptv[:])

            # ---- Phase C: attention
            with tc.tile_pool(name="attn", bufs=4) as at_p, \
                 tc.tile_pool(name="bcst", bufs=2) as bc_p, \
                 tc.tile_pool(name="rcp", bufs=2) as rc_p, \
                 tc.tile_pool(name="ps_sc", bufs=2, space="PSUM") as ps_sc, \
                 tc.tile_pool(name="ps_o", bufs=2, space="PSUM") as ps_o, \
                 tc.tile_pool(name="ps_sum", bufs=1, space="PSUM") as ps_sum, \
                 tc.tile_pool(name="ps_bc", bufs=1, space="PSUM") as ps_bc:
                for qc in range(QC):
                    kmax = (qc + 1) * 4
                    q0 = qc * 512
                    for h in range(HL):
                        po = ps_o.tile([128, 512], F32, tag="po")
                        psum = ps_sum.tile([1, 512], F32, tag="ps")
                        for kp in range(kmax // 2):
                            psc = ps_sc.tile([128, 1024], F32, tag="sc")
                            at = at_p.tile([128, 1024], F16, tag="at")
                            for half in range(2):
                                ki = kp * 2 + half
                                nc.tensor.matmul(
                                    psc[:, half * 512:(half + 1) * 512],
                                    sb_KT[:, ki * 128:(ki + 1) * 128],
                                    sb_QT[:, h, q0:q0 + 512],
                                    start=True, stop=True)
                            nc.scalar.activation(at[:], psc[:], Exp, scale=SCALE)
                            for half in range(2):
                                ki = kp * 2 + half
                                if ki >= qc * 4:
                                    nc.vector.tensor_mul(
                                        at[:, half * 512:(half + 1) * 512],
                                        at[:, half * 512:(half + 1) * 512],
                                        sb_mk[:, ki - qc * 4, :])
                            for half in range(2):
                                ki = kp * 2 + half
                                nc.tensor.matmul(
                                    po[:], sb_V[:, ki, :],
                                    at[:, half * 512:(half + 1) * 512],
                                    start=(ki == 0), stop=(ki == kmax - 1))
                                nc.tensor.matmul(
                                    psum[:], ones16[:],
                                    at[:, half * 512:(half + 1) * 512],
                                    start=(ki == 0), stop=(ki == kmax - 1))
                        rc = rc_p.tile([1, 512], F32)
                        nc.vector.reciprocal_approx_fast(rc[:], psum[:])
                        pbc = ps_bc.tile([128, 512], F32, tag="bc")
                        nc.tensor.matmul(pbc[:], ones32[:], rc[:],
                                         start=True, stop=True)
                        bc = bc_p.tile([128, 512], F32)
                        nc.vector.tensor_copy(bc[:], pbc[:])
                        nc.vector.tensor_mul(
                            sb_oT[:, h, q0:q0 + 512], po[:], bc[:])

            # ---- Phase D: O projection
            with tc.tile_pool(name="otile", bufs=4) as ot_p, \
                 tc.tile_pool(name="ps_ot", bufs=4, space="PSUM") as ps_ot:
                for oi in range(FT):
                    for qc in range(QC):
                        pot = ps_ot.tile([128, 512], F32, tag="ot")
                        for dvi in range(HL):
                            nc.tensor.matmul(
                                pot[:], sb_wo[:, dvi, oi * 128:(oi + 1) * 128],
                                sb_oT[:, dvi, qc * 512:(qc + 1) * 512],
                                start=(dvi == 0), stop=(dvi == HL - 1))
                        otc = ot_p.tile([128, 512], F32)
                        if qc % 2 == 0:
                            nc.vector.tensor_copy(otc[:], pot[:])
                        else:
                            nc.scalar.copy(otc[:], pot[:])
                        nc.sync.dma_start(
                            d_ot[oi * 128:(oi + 1) * 128,
                                 qc * 512:(qc + 1) * 512], otc[:])

    nc.compile()
    return nc


def _prep_shards(x, freqs_cos, freqs_sin, wq, wk, wv, wo):
    perm = np.empty(128, dtype=np.int64)
    perm[0:64] = 2 * np.arange(64)
    perm[64:128] = 2 * np.arange(64) + 1

    cosT = np.ascontiguousarray(freqs_cos.T).astype(np.float32)
    sinT = np.ascontiguousarray(freqs_sin.T).astype(np.float32)
    cjoin = np.concatenate([cosT, cosT], axis=0).astype(np.float16)
    sjoin = np.concatenate([sinT, -sinT], axis=0).astype(np.float16)

    masks = np.zeros((4, 128, 512), dtype=np.float16)
    q_idx = np.arange(512)[None, :]
    k_idx = np.arange(128)[:, None]
    for m in range(4):
        masks[m] = (q_idx >= m * 128 + k_idx).astype(np.float16)
    ident = np.eye(128, dtype=np.float16)

    in_maps = []
    for c in range(8):
        b, g = c // 4, c % 4
        wq_g = np.ascontiguousarray(
            wq[:, g * 512:(g + 1) * 512].reshape(DIM, 4, 128)[:, :, perm]
            .reshape(DIM, 512)).astype(np.float16)
        wk_g = np.ascontiguousarray(
            wk[:, g * 128:(g + 1) * 128][:, perm]).astype(np.float16)
        wv_g = np.ascontiguousarray(
            wv[:, g * 128:(g + 1) * 128]).astype(np.float16)
        wo_g = np.ascontiguousarray(
            wo[g * 512:(g + 1) * 512, :]).astype(np.float16)
        in_maps.append({
            "x_c": np.ascontiguousarray(x[b]).astype(np.float16),
            "wq_c": wq_g, "wk_c": wk_g, "wv_c": wv_g, "wo_c": wo_g,
            "cjoin": cjoin, "sjoin": sjoin, "masks": masks, "ident": ident,
        })
    return in_maps


def _assemble(results):
    out = np.zeros((B, S, DIM), dtype=np.float32)
    for c in range(8):
        out[c // 4] += results[c]["ot"].T
    return out


def kernel(x, freqs_cos, freqs_sin, wq, wk, wv, wo):
    x = np.asarray(x, dtype=np.float32)
    if "nc" not in _CACHE:
        _CACHE["nc"] = _build()
    nc = _CACHE["nc"]
    in_maps = _prep_shards(x, np.asarray(freqs_cos), np.asarray(freqs_sin),
                           np.asarray(wq), np.asarray(wk), np.asarray(wv),
                           np.asarray(wo))
    res = bass_utils.run_bass_kernel_spmd(nc, in_maps, core_ids=list(range(8)))
    return _assemble(res.results)



# revision 9
# speedup vs baseline: 1.0329x; 1.0329x over previous
"""Tensor-parallel GQA attention kernel for 8 Trainium2 NeuronCores.

Problem: x[2,2048,2048] -> Attention(16 q heads, 4 kv heads, rotary,
causal) -> out[2,2048,2048].

Sharding: core c handles batch b=c//4 and kv-group g=c%4 (4 q-heads +
1 kv-head). Each core computes its heads' attention output and a
partial O-projection; the host sums the 4 partials per batch.

v2 schedule (vs v1):
  - All weights are pre-arranged on the host into their exact SBUF
    images so every weight DMA is a contiguous load, issued on the
    gpsimd (SWDGE) queue in dependency-priority order.
  - x is DMA-transposed in 16 full-column tiles, round-robined across
    the sync and scalar HWDGE queues starting at t=0, so the first
    projection matmul can start at ~3us (was 38us).
  - A short burst of warm-up matmuls on a memset tile releases the PE
    HAM clock gate before real data arrives.
  - Projections run sc-chunk-outer / feature-tile-inner with 6 live
    PSUM accumulators so PE consumption tracks DMA arrival.
  - Attention is software-pipelined depth-2 (scores for k-tile ki
    issue ahead of AV/sums for ki-2) so the PE never waits on the
    scalar-engine exp.
  - Softmax normalization is fully off the PE: DVE reciprocal ->
    gpsimd partition_broadcast -> DVE multiply.
  - The O-projection for q-chunk qc is interleaved into the head
    starts of qc+1, and its fp16 tile-blocked output DMA is spread
    across the kernel instead of a serial tail phase.
"""
import numpy as np

import concourse.bass as bass
import concourse.tile as tile
import concourse.mybir as mybir
from concourse import bacc
from concourse import bass_utils

F32 = mybir.dt.float32
F16 = mybir.dt.float16

DIM = 2048
S = 2048
B = 2
HL = 4           # q heads per core
FT = DIM // 128  # feature tiles (16)
TT = S // 128    # token tiles (16)
QC = 4           # q chunks (512)
SC = 4           # projection token chunks (512)
SCALE = 1.0 / np.sqrt(128.0)

_CACHE = {}


def _build(dbg=False):
    nc = bacc.Bacc("TRN2", target_bir_lowering=False, debug=False,
                   enable_asserts=True, num_devices=8)

    d_x = nc.dram_tensor("x_c", (S, DIM), F16, kind="ExternalInput").ap()
    d_wq = nc.dram_tensor("wq_c", (128, FT * 512), F16, kind="ExternalInput").ap()
    d_wk = nc.dram_tensor("wk_c", (128, FT * 128), F16, kind="ExternalInput").ap()
    d_wv = nc.dram_tensor("wv_c", (128, FT * 128), F16, kind="ExternalInput").ap()
    d_wo = nc.dram_tensor("wo_c", (128, HL * DIM), F16, kind="ExternalInput").ap()
    d_cj = nc.dram_tensor("cjoin", (128, S), F16, kind="ExternalInput").ap()
    d_sj = nc.dram_tensor("sjoin", (128, S), F16, kind="ExternalInput").ap()
    d_mk = nc.dram_tensor("masks", (128, 4 * 512), F16, kind="ExternalInput").ap()
    d_id = nc.dram_tensor("ident", (128, 128), F16, kind="ExternalInput").ap()
    d_ot = nc.dram_tensor("ot", (FT, QC, 128, 512), F16, kind="ExternalOutput").ap()
    if dbg:
        d_dbg_qt = nc.dram_tensor("dbg_qt", (128, HL, S), F16,
                                  kind="ExternalOutput").ap()
        d_dbg_kt = nc.dram_tensor("dbg_kt", (128, S), F16,
                                  kind="ExternalOutput").ap()
        d_dbg_v = nc.dram_tensor("dbg_v", (128, TT, 128), F16,
                                 kind="ExternalOutput").ap()
        d_dbg_ot = nc.dram_tensor("dbg_oT", (128, HL, S), F16,
                                  kind="ExternalOutput").ap()

    Exp = mybir.ActivationFunctionType.Exp

    with tile.TileContext(nc) as tc:
        with tc.tile_pool(name="wts", bufs=1) as wp, \
             tc.tile_pool(name="acts", bufs=1) as ap:
            sb_x = wp.tile([128, FT, S], F16)
            sb_wq = wp.tile([128, FT, 512], F16)
            sb_wk = wp.tile([128, FT, 128], F16)
            sb_wv = wp.tile([128, FT, 128], F16)
            sb_wo = wp.tile([128, HL, DIM], F16)
            sb_cj = wp.tile([128, S], F16)
            sb_sj = wp.tile([128, S], F16)
            sb_mk = wp.tile([128, 4, 512], F16)
            sb_id = wp.tile([128, 128], F16)
            ones16 = wp.tile([128, 1], F16)
            nc.vector.memset(ones16[:], 1.0)
            warm = wp.tile([128, 512], F16)
            nc.vector.memset(warm[:], 0.125)

            sb_QT = ap.tile([128, HL, S], F16)
            sb_KT = ap.tile([128, S], F16)
            sb_V = ap.tile([128, TT, 128], F16)
            sb_oT = ap.tile([128, HL, S], F16)

            # ---- DMA kickoff: weights on the gpsimd SWDGE queue in
            # dependency order; x transposes round-robin sync/scalar.
            nc.gpsimd.dma_start(sb_wq[:, 0:4, :], d_wq[:, 0:4 * 512])
            nc.gpsimd.dma_start(sb_wk[:], d_wk)
            nc.gpsimd.dma_start(sb_wv[:], d_wv)
            nc.gpsimd.dma_start(sb_wq[:, 4:8, :], d_wq[:, 4 * 512:8 * 512])
            nc.gpsimd.dma_start(sb_wq[:, 8:12, :], d_wq[:, 8 * 512:12 * 512])
            nc.gpsimd.dma_start(sb_wq[:, 12:16, :], d_wq[:, 12 * 512:16 * 512])
            nc.gpsimd.dma_start(sb_cj[:], d_cj)
            nc.gpsimd.dma_start(sb_sj[:], d_sj)
            nc.gpsimd.dma_start(sb_id[:], d_id)
            nc.gpsimd.dma_start(sb_mk[:], d_mk)
            nc.gpsimd.dma_start(sb_wo[:], d_wo)

            # All transposes must share one queue: concurrent DMA
            # transposes on different queues corrupt each other (shared
            # XBAR). Chunked sc-outer/fi-inner so arrival order matches
            # the PE's consumption order in phase A.
            for sc in range(SC):
                for fi in range(FT):
                    nc.sync.dma_start(
                        sb_x[:, fi, sc * 512:(sc + 1) * 512],
                        d_x[sc * 512:(sc + 1) * 512,
                            fi * 128:(fi + 1) * 128],
                        transpose=True)

            # ---- Phase A: projections + RoPE, sc-outer / fi-inner
            with tc.tile_pool(name="pj", bufs=1, space="PSUM") as pj, \
                 tc.tile_pool(name="tr", bufs=2, space="PSUM") as tr, \
                 tc.tile_pool(name="vt", bufs=2) as vt_p, \
                 tc.tile_pool(name="rope", bufs=2) as rp:

                # warm up the PE HAM clock gate while DMAs land
                for _ in range(8):
                    pwarm = tr.tile([128, 512], F32, tag="tr", name="pwarm",
                                    bufs=1)
                    nc.tensor.matmul(pwarm[:], warm[:, 0:128], warm[:],
                                     start=True, stop=True)

                def rope(T, c0):
                    # T: [128, 512] fp16 chunk at token offset c0
                    mc = rp.tile([128, 512], F16, tag="mc", name="mc")
                    ms = rp.tile([128, 512], F16, tag="ms", name="ms")
                    cjs = sb_cj[:, c0:c0 + 512]
                    sjs = sb_sj[:, c0:c0 + 512]
                    nc.gpsimd.tensor_mul(mc[:], T, cjs)
                    nc.vector.tensor_mul(ms[0:64, :], T[64:128, :], sjs[64:128, :])
                    nc.vector.tensor_mul(ms[64:128, :], T[0:64, :], sjs[0:64, :])
                    nc.vector.tensor_add(T, mc[:], ms[:])

                for sc in range(SC):
                    s0 = sc * 512
                    pq = [pj.tile([128, 512], F32, tag=f"q{h}", name=f"pq{h}") for h in range(HL)]
                    pk = pj.tile([128, 512], F32, tag="k", name="pk")
                    pv = pj.tile([128, 512], F32, tag="v", name="pv")
                    for fi in range(FT):
                        xts = sb_x[:, fi, s0:s0 + 512]
                        st, sp = (fi == 0), (fi == FT - 1)
                        for h in range(HL):
                            nc.tensor.matmul(
                                pq[h][:], sb_wq[:, fi, h * 128:(h + 1) * 128],
                                xts, start=st, stop=sp)
                        nc.tensor.matmul(pk[:], sb_wk[:, fi, :], xts,
                                         start=st, stop=sp)
                        nc.tensor.matmul(pv[:], sb_wv[:, fi, :], xts,
                                         start=st, stop=sp)
                    for h in range(HL):
                        nc.vector.tensor_copy(sb_QT[:, h, s0:s0 + 512], pq[h][:])
                        rope(sb_QT[:, h, s0:s0 + 512], s0)
                    nc.vector.tensor_copy(sb_KT[:, s0:s0 + 512], pk[:])
                    rope(sb_KT[:, s0:s0 + 512], s0)
                    vt = vt_p.tile([128, 512], F16, name="vt")
                    nc.vector.tensor_copy(vt[:], pv[:])
                    for tl in range(4):
                        ti = sc * 4 + tl
                        ptv = tr.tile([128, 128], F16, tag="tr2", name="ptv",
                                      bufs=1)
                        nc.tensor.transpose(
                            ptv[:], vt[:, tl * 128:(tl + 1) * 128], sb_id[:])
                        nc.vector.tensor_copy(sb_V[:, ti, :], ptv[:])

            # ---- Phase C+D: attention with interleaved O-projection
            with tc.tile_pool(name="ps_sc", bufs=3, space="PSUM") as sc_p, \
                 tc.tile_pool(name="ps_o", bufs=2, space="PSUM") as po_p, \
                 tc.tile_pool(name="ps_sum", bufs=1, space="PSUM") as sum_p, \
                 tc.tile_pool(name="ps_ot", bufs=2, space="PSUM") as pot_p, \
                 tc.tile_pool(name="attn", bufs=3) as at_p, \
                 tc.tile_pool(name="rcp", bufs=2) as rc_p, \
                 tc.tile_pool(name="bcst", bufs=2) as bc_p, \
                 tc.tile_pool(name="otile", bufs=4) as ot_p:

                def make_oproj(qc, oi):
                    def emit():
                        pot = pot_p.tile([128, 512], F32, tag="ot", name="pot")
                        for dvi in range(HL):
                            nc.tensor.matmul(
                                pot[:], sb_wo[:, dvi, oi * 128:(oi + 1) * 128],
                                sb_oT[:, dvi, qc * 512:(qc + 1) * 512],
                                start=(dvi == 0), stop=(dvi == HL - 1))
                        otc = ot_p.tile([128, 512], F16, name="otc")
                        if qc < 2:
                            nc.scalar.copy(otc[:], pot[:])
                        else:
                            nc.vector.tensor_copy(otc[:], pot[:])
                        eng = nc.sync if oi % 2 == 0 else nc.scalar
                        eng.dma_start(d_ot[oi, qc], otc[:])
                    return emit

                pending = []
                for qc in range(QC):
                    nk = 4 * qc + 4
                    q0 = qc * 512
                    for h in range(HL):
                        for _ in range(4):
                            if pending:
                                pending.pop(0)()
                        po = po_p.tile([128, 512], F32, tag="po", name="po")
                        psum = sum_p.tile([1, 512], F32, tag="ps", name="psum")
                        at_hist = {}

                        def issue_av(ki):
                            at = at_hist.pop(ki)
                            nc.tensor.matmul(po[:], sb_V[:, ki, :], at,
                                             start=(ki == 0), stop=(ki == nk - 1))
                            nc.tensor.matmul(psum[:], ones16[:], at,
                                             start=(ki == 0), stop=(ki == nk - 1))

                        for ki in range(nk):
                            psc = sc_p.tile([128, 512], F32, tag="sc", name="psc")
                            nc.tensor.matmul(
                                psc[:], sb_KT[:, ki * 128:(ki + 1) * 128],
                                sb_QT[:, h, q0:q0 + 512], start=True, stop=True)
                            if ki >= 2:
                                issue_av(ki - 2)
                            at = at_p.tile([128, 512], F16, tag="at", name="at")
                            nc.scalar.activation(at[:], psc[:], Exp, scale=SCALE)
                            if ki >= 4 * qc:
                                nc.vector.tensor_mul(at[:], at[:],
                                                     sb_mk[:, ki - 4 * qc, :])
                            at_hist[ki] = at[:]
                        issue_av(nk - 2)
                        issue_av(nk - 1)

                        rc = rc_p.tile([1, 512], F32, name="rc")
                        nc.vector.reciprocal_approx_fast(rc[:], psum[:])
                        bc = bc_p.tile([128, 512], F32, name="bc")
                        nc.gpsimd.partition_broadcast(bc[:], rc[:], 128)
                        nc.vector.tensor_mul(
                            sb_oT[:, h, q0:q0 + 512], po[:], bc[:])
                    for oi in range(FT):
                        pending.append(make_oproj(qc, oi))
                for emit in pending:
                    emit()
                if dbg:
                    nc.sync.dma_start(d_dbg_qt, sb_QT[:])
                    nc.sync.dma_start(d_dbg_kt, sb_KT[:])
                    nc.sync.dma_start(d_dbg_v, sb_V[:])
                    nc.sync.dma_start(d_dbg_ot, sb_oT[:])

    nc.compile()
    return nc


def _prep_shards(x, freqs_cos, freqs_sin, wq, wk, wv, wo):
    perm = np.empty(128, dtype=np.int64)
    perm[0:64] = 2 * np.arange(64)
    perm[64:128] = 2 * np.arange(64) + 1

    cosT = np.ascontiguousarray(freqs_cos.T).astype(np.float32)
    sinT = np.ascontiguousarray(freqs_sin.T).astype(np.float32)
    cjoin = np.concatenate([cosT, cosT], axis=0).astype(np.float16)
    sjoin = np.concatenate([sinT, -sinT], axis=0).astype(np.float16)

    masks = np.zeros((4, 128, 512), dtype=np.float16)
    q_idx = np.arange(512)[None, :]
    k_idx = np.arange(128)[:, None]
    for m in range(4):
        masks[m] = (q_idx >= m * 128 + k_idx).astype(np.float16)
    masks_img = np.ascontiguousarray(
        masks.transpose(1, 0, 2).reshape(128, 4 * 512))
    ident = np.eye(128, dtype=np.float16)

    def sbuf_image(w, out_cols):
        # [DIM, out_cols] -> SBUF image [128, FT * out_cols]
        return np.ascontiguousarray(
            w.reshape(FT, 128, out_cols).transpose(1, 0, 2)
            .reshape(128, FT * out_cols)).astype(np.float16)

    in_maps = []
    for c in range(8):
        b, g = c // 4, c % 4
        wq_g = (wq[:, g * 512:(g + 1) * 512].reshape(DIM, 4, 128)[:, :, perm]
                .reshape(DIM, 512))
        wk_g = wk[:, g * 128:(g + 1) * 128][:, perm]
        wv_g = wv[:, g * 128:(g + 1) * 128]
        # wo rows for this group's heads -> [128, HL * DIM] image
        wo_g = np.ascontiguousarray(
            wo[g * 512:(g + 1) * 512, :].reshape(HL, 128, DIM)
            .transpose(1, 0, 2).reshape(128, HL * DIM)).astype(np.float16)
        in_maps.append({
            "x_c": np.ascontiguousarray(x[b]).astype(np.float16),
            "wq_c": sbuf_image(wq_g, 512),
            "wk_c": sbuf_image(wk_g, 128),
            "wv_c": sbuf_image(wv_g, 128),
            "wo_c": wo_g,
            "cjoin": cjoin, "sjoin": sjoin, "masks": masks_img, "ident": ident,
        })
    return in_maps


def _assemble(results):
    out = np.zeros((B, S, DIM), dtype=np.float32)
    for c in range(8):
        ot = results[c]["ot"].astype(np.float32)  # (FT, QC, 128, 512)
        out[c // 4] += ot.transpose(1, 3, 0, 2).reshape(S, DIM)
    return out


def kernel(x, freqs_cos, freqs_sin, wq, wk, wv, wo):
    x = np.asarray(x, dtype=np.float32)
    if "nc" not in _CACHE:
        _CACHE["nc"] = _build()
    nc = _CACHE["nc"]
    in_maps = _prep_shards(x, np.asarray(freqs_cos), np.asarray(freqs_sin),
                           np.asarray(wq), np.asarray(wk), np.asarray(wv),
                           np.asarray(wo))
    res = bass_utils.run_bass_kernel_spmd(nc, in_maps, core_ids=list(range(8)))
    return _assemble(res.results)


# revision 11
# speedup vs baseline: 1.4023x; 1.3576x over previous
"""Tensor-parallel GQA attention kernel for 8 Trainium2 NeuronCores.

Problem: x[2,2048,2048] -> Attention(16 q heads, 4 kv heads, rotary,
causal) -> out[2,2048,2048].

Sharding: core c handles batch b=c//4 and kv-group g=c%4 (4 q-heads +
1 kv-head). Each core computes its heads' attention output and a
partial O-projection; the host sums the 4 partials per batch.

v4 schedule:
  - x is transposed on the HOST into its feature-major SBUF image, so
    the device load is 32 plain contiguous DMAs split across the sync
    and scalar queues (no DMA-transpose XBAR, which is slow and cannot
    run concurrently with itself across queues).
  - All weights are host-packed into exact SBUF images and loaded on
    the gpsimd SWDGE queue in dependency-priority order.
  - Projections (sc chunk) and attention (same-index q-chunk) blocks
    are fully interleaved: sc0 P, qc0 A, sc1 P, qc1 A, ... so softmax
    normalization chains and the O-projection of qc overlap the next
    projection/attention block on other engines.
  - One persistent 8-bank PSUM plan: 2 rotating accumulator banks
    (shared by projection passes and O-projection groups), 3 score
    banks, 2 attention-output banks, 1 softmax-sum bank.
  - Exact-causal: diagonal k-tiles only compute q >= 128*j, a single
    shared 128x128 triangle masks the first 128 columns.
  - Attention is software-pipelined depth-3 (scores for k-tile ki
    issue ahead of AV/sums for ki-3) so the PE never waits on the
    scalar-engine exp.
  - Softmax normalization is off the PE: DVE reciprocal (fp16) ->
    gpsimd partition_broadcast -> DVE multiply.
  - V is transposed SBUF->SBUF with the (otherwise idle) XBAR.
  - Output is fp16, tile-blocked for large DMA packets, written as
    soon as each O-projection tile completes.
"""
import numpy as np

import concourse.bass as bass
import concourse.tile as tile
import concourse.mybir as mybir
from concourse import bacc
from concourse import bass_utils

F32 = mybir.dt.float32
F16 = mybir.dt.float16

DIM = 2048
S = 2048
B = 2
HL = 4           # q heads per core
FT = DIM // 128  # feature tiles (16)
TT = S // 128    # token tiles (16)
QC = 4           # q chunks (512)
SC = 4           # projection token chunks (512)
SCALE = 1.0 / np.sqrt(128.0)

_CACHE = {}


def _build(dbg=False):
    nc = bacc.Bacc("TRN2", target_bir_lowering=False, debug=False,
                   enable_asserts=True, num_devices=8)

    d_x = nc.dram_tensor("x_c", (128, FT * S), F16, kind="ExternalInput").ap()
    d_wq = nc.dram_tensor("wq_c", (128, FT * 512), F16, kind="ExternalInput").ap()
    d_wk = nc.dram_tensor("wk_c", (128, FT * 128), F16, kind="ExternalInput").ap()
    d_wv = nc.dram_tensor("wv_c", (128, FT * 128), F16, kind="ExternalInput").ap()
    d_wo = nc.dram_tensor("wo_c", (128, HL * DIM), F16, kind="ExternalInput").ap()
    d_cj = nc.dram_tensor("cjoin", (128, S), F16, kind="ExternalInput").ap()
    d_sj = nc.dram_tensor("sjoin", (128, S), F16, kind="ExternalInput").ap()
    d_mk = nc.dram_tensor("masks", (128, 128), F16, kind="ExternalInput").ap()
    d_ot = nc.dram_tensor("ot", (FT, QC, 128, 512), F16, kind="ExternalOutput").ap()
    if dbg:
        d_dbg_qt = nc.dram_tensor("dbg_qt", (128, HL, S), F16,
                                  kind="ExternalOutput").ap()
        d_dbg_kt = nc.dram_tensor("dbg_kt", (128, S), F16,
                                  kind="ExternalOutput").ap()
        d_dbg_v = nc.dram_tensor("dbg_v", (128, TT, 128), F16,
                                 kind="ExternalOutput").ap()
        d_dbg_ot = nc.dram_tensor("dbg_oT", (128, HL, S), F16,
                                  kind="ExternalOutput").ap()

    Exp = mybir.ActivationFunctionType.Exp

    with tile.TileContext(nc) as tc:
        with tc.tile_pool(name="wts", bufs=1) as wp, \
             tc.tile_pool(name="acts", bufs=1) as ap, \
             tc.tile_pool(name="vt", bufs=2) as vt_p, \
             tc.tile_pool(name="rope", bufs=2) as rp, \
             tc.tile_pool(name="attn", bufs=4) as at_p, \
             tc.tile_pool(name="rcp", bufs=2) as rc_p, \
             tc.tile_pool(name="bcst", bufs=2) as bc_p, \
             tc.tile_pool(name="otile", bufs=4) as ot_p, \
             tc.tile_pool(name="psA", bufs=1, space="PSUM") as psA, \
             tc.tile_pool(name="ps_sc", bufs=3, space="PSUM") as sc_p, \
             tc.tile_pool(name="ps_o", bufs=2, space="PSUM") as po_p, \
             tc.tile_pool(name="ps_sum", bufs=1, space="PSUM") as sum_p:

            sb_x = wp.tile([128, FT, S], F16)
            sb_wq = wp.tile([128, FT, 512], F16)
            sb_wk = wp.tile([128, FT, 128], F16)
            sb_wv = wp.tile([128, FT, 128], F16)
            sb_wo = wp.tile([128, HL, DIM], F16)
            sb_cj = wp.tile([128, S], F16)
            sb_sj = wp.tile([128, S], F16)
            sb_mk = wp.tile([128, 128], F16)
            ones16 = wp.tile([128, 1], F16)
            nc.vector.memset(ones16[:], 1.0)
            warm = wp.tile([128, 512], F16)
            nc.vector.memset(warm[:], 0.125)

            sb_QT = ap.tile([128, HL, S], F16)
            sb_KT = ap.tile([128, S], F16)
            sb_V = ap.tile([128, TT, 128], F16)
            sb_oT = ap.tile([128, HL, S], F16)

            # ---- DMA kickoff
            # x halves: [fi, half] contiguous 2KB/partition chunks,
            # alternating sync/scalar queues; first-needed first.
            for half in range(2):
                for fi in range(FT):
                    eng = nc.sync if fi % 2 == 0 else nc.scalar
                    eng.dma_start(
                        sb_x[:, fi, half * 1024:(half + 1) * 1024],
                        d_x[:, fi * S + half * 1024:fi * S + half * 1024 + 1024])
            # weights on the gpsimd SWDGE queue in dependency order
            nc.gpsimd.dma_start(sb_wq[:, 0:4, :], d_wq[:, 0:4 * 512])
            nc.gpsimd.dma_start(sb_wk[:], d_wk)
            nc.gpsimd.dma_start(sb_wv[:], d_wv)
            nc.gpsimd.dma_start(sb_wq[:, 4:8, :], d_wq[:, 4 * 512:8 * 512])
            nc.gpsimd.dma_start(sb_wq[:, 8:12, :], d_wq[:, 8 * 512:12 * 512])
            nc.gpsimd.dma_start(sb_wq[:, 12:16, :], d_wq[:, 12 * 512:16 * 512])
            nc.gpsimd.dma_start(sb_cj[:], d_cj)
            nc.gpsimd.dma_start(sb_sj[:], d_sj)
            nc.gpsimd.dma_start(sb_mk[:], d_mk)
            nc.gpsimd.dma_start(sb_wo[:], d_wo)

            # warm up the PE HAM clock gate while the first DMAs land
            for w in range(8):
                pwarm = psA.tile([128, 512], F32, tag="a", name="pwarm")
                nc.tensor.matmul(pwarm[:], warm[:, 0:128], warm[:],
                                 start=True, stop=True)

            def rope(T, c0):
                # T: [128, 512] fp16 chunk at token offset c0
                mc = rp.tile([128, 512], F16, tag="mc", name="mc")
                ms = rp.tile([128, 512], F16, tag="ms", name="ms")
                cjs = sb_cj[:, c0:c0 + 512]
                sjs = sb_sj[:, c0:c0 + 512]
                nc.gpsimd.tensor_mul(mc[:], T, cjs)
                nc.vector.tensor_mul(ms[0:64, :], T[64:128, :], sjs[64:128, :])
                nc.vector.tensor_mul(ms[64:128, :], T[0:64, :], sjs[0:64, :])
                nc.vector.tensor_add(T, mc[:], ms[:])

            def proj_block(sc):
                s0 = sc * 512
                # 3 passes of 2 interleaved accumulations (jobs: q0..q3, k, v)
                passes = [[("q", 0), ("q", 1)], [("q", 2), ("k", 0)],
                          [("q", 3), ("v", 0)]]
                for pair in passes:
                    accs = []
                    for t, (kind, idx) in zip("ab", pair):
                        accs.append(psA.tile([128, 512], F32, tag=t,
                                             name=f"acc_{kind}{idx}"))
                    for fi in range(FT):
                        xts = sb_x[:, fi, s0:s0 + 512]
                        st, sp = (fi == 0), (fi == FT - 1)
                        for acc, (kind, idx) in zip(accs, pair):
                            if kind == "q":
                                w = sb_wq[:, fi, idx * 128:(idx + 1) * 128]
                            elif kind == "k":
                                w = sb_wk[:, fi, :]
                            else:
                                w = sb_wv[:, fi, :]
                            nc.tensor.matmul(acc[:], w, xts, start=st, stop=sp)
                    for acc, (kind, idx) in zip(accs, pair):
                        if kind == "q":
                            nc.vector.tensor_copy(sb_QT[:, idx, s0:s0 + 512],
                                                  acc[:])
                            rope(sb_QT[:, idx, s0:s0 + 512], s0)
                        elif kind == "k":
                            nc.vector.tensor_copy(sb_KT[:, s0:s0 + 512], acc[:])
                            rope(sb_KT[:, s0:s0 + 512], s0)
                        else:
                            vt = vt_p.tile([128, 512], F16, name="vt")
                            nc.vector.tensor_copy(vt[:], acc[:])
                            for tl in range(4):
                                nc.sync.dma_start(
                                    sb_V[:, sc * 4 + tl, :],
                                    vt[:, tl * 128:(tl + 1) * 128],
                                    transpose=True)

            pending = []

            def make_oproj(qc, oi):
                def emit():
                    pot = psA.tile([128, 512], F32, tag=("a" if oi % 2 else "b"),
                                   name="pot")
                    for dvi in range(HL):
                        nc.tensor.matmul(
                            pot[:], sb_wo[:, dvi, oi * 128:(oi + 1) * 128],
                            sb_oT[:, dvi, qc * 512:(qc + 1) * 512],
                            start=(dvi == 0), stop=(dvi == HL - 1))
                    otc = ot_p.tile([128, 512], F16, name="otc")
                    nc.vector.tensor_copy(otc[:], pot[:])
                    eng = nc.sync if oi % 2 == 0 else nc.scalar
                    eng.dma_start(d_ot[oi, qc], otc[:])
                return emit

            def attn_head(qc, h):
                nk = 4 * qc + 4
                q0 = qc * 512
                for _ in range(4):
                    if pending:
                        pending.pop(0)()
                po = po_p.tile([128, 512], F32, tag="po", name="po")
                ps = sum_p.tile([1, 512], F32, tag="ps", name="ps")
                info = {}

                def issue_av(ki):
                    at, off, wd = info.pop(ki)
                    nc.tensor.matmul(po[:, off:512], sb_V[:, ki, :], at,
                                     start=(ki == 0), stop=(ki == nk - 1))
                    nc.tensor.matmul(ps[:, off:512], ones16[:], at,
                                     start=(ki == 0), stop=(ki == nk - 1))

                for ki in range(nk):
                    j = ki - 4 * qc
                    off = 128 * j if j > 0 else 0
                    wd = 512 - off
                    psc = sc_p.tile([128, 512], F32, tag="sc", name="psc")
                    nc.tensor.matmul(
                        psc[:, 0:wd], sb_KT[:, ki * 128:(ki + 1) * 128],
                        sb_QT[:, h, q0 + off:q0 + 512], start=True, stop=True)
                    if ki >= 3:
                        issue_av(ki - 3)
                    at = at_p.tile([128, 512], F16, tag="at", name="at")
                    nc.scalar.activation(at[:, 0:wd], psc[:, 0:wd], Exp,
                                         scale=SCALE)
                    if j >= 0:
                        nc.vector.tensor_mul(at[:, 0:128], at[:, 0:128],
                                             sb_mk[:])
                    info[ki] = (at[:, 0:wd], off, wd)
                for ki in range(max(0, nk - 3), nk):
                    if ki in info:
                        issue_av(ki)

                rc = rc_p.tile([1, 512], F32, name="rc")
                nc.vector.reciprocal_approx_fast(rc[:], ps[:])
                rch = rc_p.tile([1, 512], F16, tag="rch", name="rch")
                nc.vector.tensor_copy(rch[:], rc[:])
                bc = bc_p.tile([128, 512], F16, name="bc")
                nc.gpsimd.partition_broadcast(bc[:], rch[:], 128)
                nc.vector.tensor_mul(sb_oT[:, h, q0:q0 + 512], po[:], bc[:])

            for blk in range(SC):
                proj_block(blk)
                for h in range(HL):
                    attn_head(blk, h)
                for oi in range(FT):
                    pending.append(make_oproj(blk, oi))
            for emit in pending:
                emit()

            if dbg:
                nc.sync.dma_start(d_dbg_qt, sb_QT[:])
                nc.sync.dma_start(d_dbg_kt, sb_KT[:])
                nc.sync.dma_start(d_dbg_v, sb_V[:])
                nc.sync.dma_start(d_dbg_ot, sb_oT[:])

    nc.compile()
    return nc


def _prep_shards(x, freqs_cos, freqs_sin, wq, wk, wv, wo):
    perm = np.empty(128, dtype=np.int64)
    perm[0:64] = 2 * np.arange(64)
    perm[64:128] = 2 * np.arange(64) + 1

    cosT = np.ascontiguousarray(freqs_cos.T).astype(np.float32)
    sinT = np.ascontiguousarray(freqs_sin.T).astype(np.float32)
    cjoin = np.concatenate([cosT, cosT], axis=0).astype(np.float16)
    sjoin = np.concatenate([sinT, -sinT], axis=0).astype(np.float16)

    # single causal triangle for the first 128 columns of diagonal tiles
    q_idx = np.arange(128)[None, :]
    k_idx = np.arange(128)[:, None]
    mask128 = (q_idx >= k_idx).astype(np.float16)

    def sbuf_image(w, out_cols):
        # [DIM, out_cols] -> SBUF image [128, FT * out_cols]
        return np.ascontiguousarray(
            w.reshape(FT, 128, out_cols).transpose(1, 0, 2)
            .reshape(128, FT * out_cols)).astype(np.float16)

    in_maps = []
    for c in range(8):
        b, g = c // 4, c % 4
        x_img = sbuf_image(np.ascontiguousarray(x[b].T).astype(np.float16)
                           .reshape(DIM, S), S)
        wq_g = (wq[:, g * 512:(g + 1) * 512].reshape(DIM, 4, 128)[:, :, perm]
                .reshape(DIM, 512))
        wk_g = wk[:, g * 128:(g + 1) * 128][:, perm]
        wv_g = wv[:, g * 128:(g + 1) * 128]
        wo_g = np.ascontiguousarray(
            wo[g * 512:(g + 1) * 512, :].reshape(HL, 128, DIM)
            .transpose(1, 0, 2).reshape(128, HL * DIM)).astype(np.float16)
        in_maps.append({
            "x_c": x_img,
            "wq_c": sbuf_image(wq_g, 512),
            "wk_c": sbuf_image(wk_g, 128),
            "wv_c": sbuf_image(wv_g, 128),
            "wo_c": wo_g,
            "cjoin": cjoin, "sjoin": sjoin, "masks": mask128,
        })
    return in_maps


def _assemble(results):
    out = np.zeros((B, S, DIM), dtype=np.float32)
    for c in range(8):
        ot = results[c]["ot"].astype(np.float32)  # (FT, QC, 128, 512)
        out[c // 4] += ot.transpose(1, 3, 0, 2).reshape(S, DIM)
    return out


def kernel(x, freqs_cos, freqs_sin, wq, wk, wv, wo):
    x = np.asarray(x, dtype=np.float32)
    if "nc" not in _CACHE:
        _CACHE["nc"] = _build()
    nc = _CACHE["nc"]
    in_maps = _prep_shards(x, np.asarray(freqs_cos), np.asarray(freqs_sin),
                           np.asarray(wq), np.asarray(wk), np.asarray(wv),
                           np.asarray(wo))
    res = bass_utils.run_bass_kernel_spmd(nc, in_maps, core_ids=list(range(8)))
    return _assemble(res.results)
